# revision 9
# baseline (speedup 1.0000x reference)
"""Trainium2 Bass kernel for the linear-attention block (nn_Attention).

Per batch element (x: [256, 4096] after flattening h*w):
    qkv = w_qkv @ x; q,k,v heads of 64
    q = softmax_d(q) * 64**-0.5 ; k = softmax_n(k) ; v = v/4096
    ctx[h] = k[h] @ v[h].T ; out[h] = ctx[h].T @ q[h]
    y = w_out @ out + b_out ; LayerNorm_c(y) * g
Sharding: data-parallel over batch, 2 batch elements per core, no collectives.

Design (v5) — elementwise-bound rework of v4:
- kv computed TRANSPOSED per 128-token chunk in ONE PSUM bank (k||v, two
  N=512 matmuls); kexp via ACT Exp, vT via DVE copy. ksum rides as ones
  columns in the vT tiles.
- q natural; per-(head,n) colsums packed into ONE [128,512] PSUM bank via
  col-offset tile_position matmuls -> a single reciprocal_approx_fast +
  cast for 4 n-tiles (instead of 16 tiny [4,512] DVE ops).
- qbc broadcast via row-offset tile_position matmuls against the packed
  qsr bank; qn = qexp * qbc on DVE.
- stage C: y stays in PSUM until scaled. ACT Square drains y^2 to scratch
  (one [128,512] op per bank), DVE tensor_scalar+accum_out builds the
  variance columns, rstd = Exp(-0.5*Ln(var+eps)) on ACT (stays in the
  natural_log_exp table family -> NO ACT table reloads anywhere), final
  scale = DVE tensor_scalar reading PSUM directly into the DMA staging
  tile. Eliminates the separate y drain pass of v4.
- x loaded in 8 ordered 512-token pieces per batch on the (idle) GpSimd
  queue so the first kv matmul starts ~1.5us in, not 13us.
- PSUM budget (8 banks): pbig 3 {kv,q} + pc 2 {yT2,G} + pqs 2 {qs,qbc}
  + pctx 1 {ctx}.
- Host folds (exact): v/n into w_v; q-scale + LN mean-centering into w_out.
"""

import numpy as np

HEADS = 4
DIM_HEAD = 64
SCALE = DIM_HEAD ** -0.5
EPS = 1e-5
B, C, H, W = 16, 256, 64, 64
N = H * W  # 4096
HID = HEADS * DIM_HEAD  # 256
NCORES = 8
BPC = B // NCORES  # batches per core = 2

NCH = N // 128   # 32 token chunks of 128
NT = N // 512    # 8 n-tiles of 512 for the q stage
CT = C // 128    # 2 contraction tiles

_cache = {}


def _build_nc(G_IS_FULL=False):
    import concourse.bass as bass
    import concourse.tile as tile
    from concourse import bacc, masks, mybir

    f32 = mybir.dt.float32
    bf16 = mybir.dt.bfloat16
    AF = mybir.ActivationFunctionType
    OP = mybir.AluOpType

    nc = bacc.Bacc(None, target_bir_lowering=False, debug=False)
    x_ext = nc.declare_dram_parameter("x", [BPC, 128, CT, N], bf16, isOutput=False)
    wqkvT_ext = nc.declare_dram_parameter("wqkvT", [128, CT, 3 * HID], bf16, isOutput=False)
    woutcT_ext = nc.declare_dram_parameter("woutcT", [128, CT, C], bf16, isOutput=False)
    bc4_ext = nc.declare_dram_parameter("bc4", [1, C], bf16, isOutput=False)
    g_ext = nc.declare_dram_parameter("g", [1, C], bf16, isOutput=False)
    # row-selector patterns for the qbc broadcast matmuls (per qt), rows at
    # 32-offsets matching the packed qsr bank
    obc2_ext = nc.declare_dram_parameter("obc2", [128, 2, 128], bf16, isOutput=False)
    out_ext = nc.declare_dram_parameter("out", [BPC, N, C], bf16, isOutput=True)

    with tile.TileContext(nc) as tc:
        with (
            tc.tile_pool(name="wts", bufs=1) as wts,
            tc.tile_pool(name="xs", bufs=2) as xs_pool,
            tc.tile_pool(name="kex", bufs=6) as kex_pool,
            tc.tile_pool(name="qb", bufs=1) as qb_pool,
            tc.tile_pool(name="qnb", bufs=2) as qn_pool,
            tc.tile_pool(name="small", bufs=3) as small_pool,
            tc.tile_pool(name="scr", bufs=4) as scr_pool,
            tc.tile_pool(name="qsr", bufs=2) as qsr_pool,
            tc.tile_pool(name="stg", bufs=3) as stg_pool,
            tc.tile_pool(name="pbig", bufs=3, space="PSUM") as pbig,
            tc.tile_pool(name="pc", bufs=2, space="PSUM") as pc_pool,
            tc.tile_pool(name="pqs", bufs=2, space="PSUM") as pqs,
            tc.tile_pool(name="pctx", bufs=1, space="PSUM") as pctx,
        ):
            # ---- constants & weights (loaded once, sync queue) ----
            wqkvT3 = wts.tile([128, CT, 3 * HID], bf16, tag="wqkvT", name="wqkvT")
            nc.sync.dma_start(out=wqkvT3, in_=wqkvT_ext[:, :, :])
            woutcT3 = wts.tile([128, CT, C], bf16, tag="woutcT", name="woutcT")
            nc.sync.dma_start(out=woutcT3, in_=woutcT_ext[:, :, :])
            bc4_sb = wts.tile([1, C], bf16, tag="bc4", name="bc4")
            nc.sync.dma_start(out=bc4_sb, in_=bc4_ext[:, :])
            g_row = wts.tile([1, C], bf16, tag="grow", name="grow")
            nc.sync.dma_start(out=g_row, in_=g_ext[:, :])
            obc2sb = wts.tile([128, 2, 128], bf16, tag="obc2", name="obc2")
            nc.sync.dma_start(out=obc2sb, in_=obc2_ext[:, :, :])

            ones1x128 = wts.tile([1, 128], bf16, tag="ones1x128", name="ones1x128")
            nc.vector.memset(ones1x128, 1.0)
            # qs lhsT pair: col (2qt+hh) sums partitions hh*64..hh*64+63
            onesblk4 = []
            for which in range(2):
                ob = wts.tile([128, 4], bf16, tag=f"onesblk4{which}", name=f"onesblk4{which}")
                nc.vector.memset(ob, 0.0)
                nc.vector.memset(ob[0:64, 2 * which:2 * which + 1], 1.0)
                nc.vector.memset(ob[64:128, 2 * which + 1:2 * which + 2], 1.0)
                onesblk4.append(ob)
            eps_sb = wts.tile([128, 1], f32, tag="eps", name="eps")
            nc.vector.memset(eps_sb, EPS)

            ident = wts.tile([128, 128], bf16, tag="ident", name="ident")
            g_bc = wts.tile([128, C], bf16, tag="gbc", name="gbc")
            late = {"done": False}

            def build_late_consts():
                # deferred so the startup DMAs/PE aren't blocked by them
                if late["done"]:
                    return
                late["done"] = True
                masks.make_identity(nc, ident[:, :])
                if G_IS_FULL:
                    g_ps = pc_pool.tile([128, 512], f32, tag="c", name="gps")
                    nc.tensor.matmul(g_ps[:, 0:C], ones1x128, g_row,
                                     start=True, stop=True)
                    nc.vector.tensor_copy(out=g_bc, in_=g_ps[:, 0:C])

            # manual vT buffers holding 2 chunks each, with persistent ones
            # columns at 128/257/386/515 (the ksum column per head-pair)
            NVT = 4
            vts = []
            for i in range(NVT):
                vt = wts.tile([128, 516], bf16, tag=f"vt{i}", name=f"vt{i}")
                for j in range(4):
                    nc.vector.memset(vt[:, j * 129 + 128:j * 129 + 129], 1.0)
                vts.append(vt)

            # ---- load x: 8 ordered 512-token pieces per batch on the gpsimd
            # queue; piece 0 of batch 0 lands first so kv matmuls start early
            xs_all = []
            for b in range(BPC):
                xs3 = xs_pool.tile([128, CT, N], bf16, tag="x", name="x")
                xr = x_ext[b]
                for p8 in range(8):
                    psl = slice(p8 * 512, (p8 + 1) * 512)
                    nc.gpsimd.dma_start(out=xs3[:, :, psl], in_=xr[:, :, psl])
                xs_all.append([xs3[:, i] for i in range(CT)])

            # per-batch state carried between stages
            st = [dict() for _ in range(BPC)]

            def stage_AB(b, c_gen=None, at_cp1=None, at_end_ctx=None):
                """Interleaved kv+ctx (transposed) and q (natural) stages."""
                xs = xs_all[b]
                ctx_t = pctx.tile([128, 258], f32, tag="ctx", name="ctx")
                st[b]["ctx"] = ctx_t
                kexps = [None] * NCH
                qexp = [qb_pool.tile([128, N], bf16, tag=f"qexp{qt}", name=f"qexp{qt}")
                        for qt in range(2)]
                qn = [qn_pool.tile([128, N], bf16, tag=f"qn{qt}", name=f"qn{qt}")
                      for qt in range(2)]
                st[b]["qn"] = qn
                qsrs = [None, None]

                def kv_chunk(ch):
                    kv_ps = pbig.tile([128, 512], f32, tag="big", name="kv")
                    csl = slice(ch * 128, (ch + 1) * 128)
                    for ct in range(CT):
                        nc.tensor.matmul(
                            kv_ps, xs[ct][:, csl], wqkvT3[:, ct, HID:3 * HID],
                            start=(ct == 0), stop=(ct == CT - 1),
                            skip_group_check=True,
                        )
                    kexp = kex_pool.tile([128, 256], bf16, tag="kexp", name="kexp")
                    nc.scalar.activation(out=kexp, in_=kv_ps[:, 0:256], func=AF.Exp)
                    kexps[ch] = kexp
                    vt = vts[(ch // 2) % NVT]
                    vo = (ch % 2) * 258
                    vdst = vt[:, vo:vo + 258].rearrange("p (hp x) -> p hp x", hp=2)[:, :, 0:128]
                    vsrc = kv_ps[:, 256:512].rearrange("p (hp x) -> p hp x", hp=2)
                    nc.vector.tensor_copy(out=vdst, in_=vsrc)

                def ctx_mms(ch):
                    kex = kexps[ch]
                    vt = vts[(ch // 2) % NVT]
                    vo = (ch % 2) * 258
                    for hp in range(2):
                        nc.tensor.matmul(
                            ctx_t[:, hp * 129:(hp + 1) * 129],
                            kex[:, hp * 128:(hp + 1) * 128],
                            vt[:, vo + hp * 129:vo + (hp + 1) * 129],
                            start=(ch == 0 and hp == 0),
                            stop=(ch == NCH - 1 and hp == 1),
                            skip_group_check=True,
                        )

                def q_mms(nt):
                    nsl = slice(nt * 512, (nt + 1) * 512)
                    for qt in range(2):
                        q_ps = pbig.tile([128, 512], f32, tag="big", name="q")
                        for ct in range(CT):
                            nc.tensor.matmul(
                                q_ps,
                                wqkvT3[:, ct, qt * 128:(qt + 1) * 128],
                                xs[ct][:, nsl],
                                start=(ct == 0), stop=(ct == CT - 1),
                            )
                        nc.scalar.activation(out=qexp[qt][:, nsl], in_=q_ps,
                                             func=AF.Exp)

                def qs_burst(g):
                    """Pack colsums for n-tiles 4g..4g+3 into one PSUM bank at
                    32-partition offsets (col-offset tile_position), then one
                    reciprocal + one bf16 cast for the whole group."""
                    qs32 = pqs.tile([128, 512], f32, tag="qsb", name="qs32")
                    for i in range(4):
                        nt = 4 * g + i
                        nsl = slice(nt * 512, (nt + 1) * 512)
                        for qt in range(2):
                            nc.tensor.matmul(
                                qs32[32 * i:32 * i + 4, :],
                                onesblk4[qt], qexp[qt][:, nsl],
                                start=(qt == 0), stop=(qt == 1),
                                skip_group_check=True,
                                tile_position=(0, 32 * i),
                            )
                    qsrf = qsr_pool.tile([128, 512], f32, tag="qsrf", name="qsrf")
                    nc.vector.reciprocal_approx_fast(out=qsrf, in_=qs32)
                    qsr = qsr_pool.tile([128, 512], bf16, tag="qsr", name="qsr")
                    nc.vector.tensor_copy(out=qsr, in_=qsrf)
                    qsrs[g] = qsr

                def qbc_qn(nt):
                    g, i = nt // 4, nt % 4
                    qsr = qsrs[g]
                    nsl = slice(nt * 512, (nt + 1) * 512)
                    for qt in range(2):
                        qbc_ps = pqs.tile([128, 512], f32, tag="qsb", name="qbc")
                        nc.tensor.matmul(
                            qbc_ps,
                            obc2sb[32 * i:32 * i + 4, qt, :],
                            qsr[32 * i:32 * i + 4, :],
                            start=True, stop=True,
                            tile_position=(32 * i, 0),
                        )
                        nc.vector.tensor_mul(out=qn[qt][:, nsl],
                                             in0=qexp[qt][:, nsl], in1=qbc_ps)

                # software-pipelined emission
                for ch in range(NCH):
                    kv_chunk(ch)
                    if ch % 4 == 3:
                        q_mms(ch // 4)
                    if ch == 1 and at_cp1 is not None:
                        at_cp1()
                    if ch >= 2:
                        ctx_mms(ch - 2)
                    if ch == 17:
                        qs_burst(0)
                    if ch >= 19 and ch % 3 == 1 and (ch - 19) // 3 < 4:
                        qbc_qn((ch - 19) // 3)
                    if c_gen is not None and ch >= 4 and ch % 2 == 1:
                        next(c_gen, None)
                ctx_mms(NCH - 2)
                ctx_mms(NCH - 1)
                if at_end_ctx is not None:
                    at_end_ctx()
                qs_burst(1)
                for nt in range(4, 8):
                    qbc_qn(nt)

            def stage_G_pre(b):
                """ctx normalize on DVE (krecip + block-diag scale)."""
                ctx_t = st[b]["ctx"]
                krecip = small_pool.tile([128, 2], f32, tag="krecip", name="krecip")
                for hp in range(2):
                    nc.vector.reciprocal(out=krecip[:, hp:hp + 1],
                                         in_=ctx_t[:, hp * 129 + 128:hp * 129 + 129])
                ctx_sb = [small_pool.tile([128, 128], bf16, tag=f"ctxsb{i}", name=f"ctxsb{i}")
                          for i in range(2)]
                for hp in range(2):
                    nc.vector.memset(ctx_sb[hp], 0.0)
                    for hh in range(2):
                        s = slice(hh * 64, hh * 64 + 64)
                        nc.vector.tensor_scalar(
                            out=ctx_sb[hp][s, s],
                            in0=ctx_t[s, hp * 129 + hh * 64:hp * 129 + hh * 64 + 64],
                            scalar1=krecip[s, hp:hp + 1],
                            scalar2=None,
                            op0=OP.mult,
                        )
                st[b]["ctx_sb"] = ctx_sb

            def stage_G_fin(b):
                """transpose + G = ctxT @ woutT (+ bias fold) on the PE."""
                ctx_sb = st[b]["ctx_sb"]
                G_sb = small_pool.tile([128, 2, C], bf16, tag="G", name="G")
                st[b]["G"] = G_sb
                for hp in range(2):
                    ctxT_ps = pc_pool.tile([128, 512], f32, tag="c", name="ctxT")
                    ctxT_ps_bf = ctxT_ps.bitcast(bf16)[:, 0:128]
                    nc.tensor.transpose(ctxT_ps_bf, ctx_sb[hp], ident)
                    ctxT_sb = small_pool.tile([128, 128], bf16, tag=f"ctxT{hp}", name=f"ctxT{hp}")
                    nc.vector.tensor_copy(out=ctxT_sb, in_=ctxT_ps_bf)
                    G_ps = pc_pool.tile([128, 512], f32, tag="c", name="Gps")
                    nc.tensor.matmul(G_ps[:, 0:C], ctxT_sb, woutcT3[:, hp],
                                     start=True, stop=False)
                    nc.tensor.matmul(G_ps[:, 0:C], ones1x128, bc4_sb,
                                     start=False, stop=True)
                    nc.vector.tensor_copy(out=G_sb[:, hp], in_=G_ps[:, 0:C])

            def stage_C(b):
                """yT = qn^T-chunks @ G (2 chunks per bank); y held in PSUM
                until the LN scale: ACT Square drains y^2, DVE accumulates
                variance columns, rstd = Exp(-0.5 Ln(var+eps)) on ACT, final
                scale reads PSUM straight into the bf16 staging tile."""
                qn = st[b]["qn"]
                G_sb = st[b]["G"]
                s2_all = small_pool.tile([128, NCH], f32, tag="s2", name="s2")
                rstd_all = small_pool.tile([128, NCH], f32, tag="rstd", name="rstd")
                outr = out_ext[b].rearrange("(c p) f -> p c f", p=128)
                stg = None
                for g2 in range(NCH // 2):
                    ch0 = g2 * 2
                    if ch0 % 8 == 0:
                        stg = stg_pool.tile([128, 8, C], bf16, tag="stg", name="stg")
                    yT2_ps = pc_pool.tile([128, 512], f32, tag="c", name="yT2")
                    for half in range(2):
                        ch = ch0 + half
                        csl = slice(ch * 128, (ch + 1) * 128)
                        for qt in range(2):
                            nc.tensor.matmul(
                                yT2_ps[:, half * C:(half + 1) * C],
                                qn[qt][:, csl], G_sb[:, qt],
                                start=(qt == 0), stop=(qt == 1),
                                skip_group_check=True,
                            )
                    # y^2 drain (ACT, one [128,512] op, exp-family table)
                    y2_sb = scr_pool.tile([128, 512], bf16, tag="y2", name="y2")
                    nc.scalar.activation(out=y2_sb, in_=yT2_ps, func=AF.Square)
                    # variance columns (DVE 4x-mode tensor_scalar + accum)
                    for half in range(2):
                        ch = ch0 + half
                        garb = scr_pool.tile([128, C], bf16, tag="garb", name="garb")
                        nc.vector.tensor_scalar(
                            out=garb, in0=y2_sb[:, half * C:(half + 1) * C],
                            scalar1=1.0, scalar2=None, op0=OP.mult,
                            op1=OP.add,
                            accum_out=s2_all[:, ch:ch + 1],
                        )
                    # rstd = exp(-0.5 * ln(s2/C + eps))  (no table switch)
                    lnv = scr_pool.tile([128, 2], f32, tag="lnv", name="lnv")
                    nc.scalar.activation(out=lnv, in_=s2_all[:, ch0:ch0 + 2],
                                         func=AF.Ln, bias=eps_sb, scale=1.0 / C)
                    nc.scalar.activation(out=rstd_all[:, ch0:ch0 + 2], in_=lnv,
                                         func=AF.Exp, scale=-0.5)
                    # final scale straight from PSUM -> staging (frees bank)
                    for half in range(2):
                        ch = ch0 + half
                        nc.vector.tensor_scalar(
                            out=stg[:, ch % 8],
                            in0=yT2_ps[:, half * C:(half + 1) * C],
                            scalar1=rstd_all[:, ch:ch + 1], scalar2=None,
                            op0=OP.mult)
                        if G_IS_FULL:
                            nc.gpsimd.tensor_mul(out=stg[:, ch % 8],
                                                 in0=stg[:, ch % 8], in1=g_bc)
                    if ch0 % 4 == 2:
                        g4 = ch0 // 4
                        nc.sync.dma_start(
                            out=outr[:, g4 * 4:(g4 + 1) * 4],
                            in_=stg[:, (g4 % 2) * 4:(g4 % 2) * 4 + 4])
                    yield

            # emission order: both batches' matmul-dense stages back-to-back
            stage_AB(0)
            build_late_consts()
            stage_G_pre(0)
            c0 = stage_C(0)
            stage_AB(1, c_gen=c0, at_cp1=lambda: stage_G_fin(0))
            stage_G_pre(1)
            for _ in c0:
                pass
            stage_G_fin(1)
            for _ in stage_C(1):
                pass

    nc.compile()
    return nc


def _prep_weights(w_qkv, w_out, b_out, g):
    import ml_dtypes
    w_qkv = np.asarray(w_qkv, dtype=np.float64)
    w_out = np.asarray(w_out, dtype=np.float64)
    b_out = np.asarray(b_out, dtype=np.float64)
    g64 = np.asarray(g, dtype=np.float64)
    wq = w_qkv.copy()
    wq[2 * HID:3 * HID, :] /= N          # fold v/n
    wqkvT = wq.T.reshape(CT, 128, 3 * HID).transpose(1, 0, 2)
    wqkvT = np.ascontiguousarray(wqkvT).astype(ml_dtypes.bfloat16)
    wo = w_out * SCALE                    # fold q scale
    wo = wo - wo.mean(axis=0, keepdims=True)  # fold LN mean-centering
    woutcT = wo.T.reshape(CT, 128, C).transpose(1, 0, 2)
    woutcT = np.ascontiguousarray(woutcT).astype(ml_dtypes.bfloat16)
    bc4 = ((b_out - b_out.mean()) / 4.0).astype(ml_dtypes.bfloat16).reshape(1, C)
    g_row = g64.astype(ml_dtypes.bfloat16).reshape(1, C)
    return wqkvT, woutcT, bc4, g_row


def _make_in_maps(x, w_qkv, w_out, b_out, g):
    import ml_dtypes
    xf = np.asarray(x, dtype=np.float32).reshape(B, CT, 128, N).transpose(0, 2, 1, 3)
    xf = np.ascontiguousarray(xf).astype(ml_dtypes.bfloat16)
    wqkvT, woutcT, bc4, g_row = _prep_weights(w_qkv, w_out, b_out, g)
    # qbc row-selector: obc2[32*i + 2*qt + hh, qt, 64*hh : 64*hh+64] = 1
    obc2 = np.zeros((128, 2, 128), dtype=ml_dtypes.bfloat16)
    for qt in range(2):
        for i in range(4):
            for hh in range(2):
                obc2[32 * i + 2 * qt + hh, qt, 64 * hh:64 * hh + 64] = 1.0
    in_maps = []
    for i in range(NCORES):
        in_maps.append({
            "x": np.ascontiguousarray(xf[i * BPC:(i + 1) * BPC]),
            "wqkvT": wqkvT,
            "woutcT": woutcT,
            "bc4": bc4,
            "g": g_row,
            "obc2": obc2,
        })
    return in_maps


def kernel(x, w_qkv, w_out, b_out, g):
    from concourse.bass_utils import run_bass_kernel_spmd

    g_full = not np.allclose(np.asarray(g, dtype=np.float64), 1.0)
    key = f"nc{int(g_full)}"
    if key not in _cache:
        _cache[key] = _build_nc(G_IS_FULL=g_full)
    nc = _cache[key]

    in_maps = _make_in_maps(x, w_qkv, w_out, b_out, g)
    res = run_bass_kernel_spmd(nc, in_maps, core_ids=list(range(NCORES)))
    outs = [res.results[i]["out"] for i in range(NCORES)]
    yT = np.concatenate(outs, axis=0).astype(np.float32)  # [B, N, C]
    y = np.ascontiguousarray(yT.transpose(0, 2, 1)).reshape(B, C, H, W)
    return y


# revision 10
# speedup vs baseline: 1.4710x; 1.4710x over previous
"""Trainium2 Bass kernel for the linear-attention block (nn_Attention).

Per batch element (x: [256, 4096] after flattening h*w):
    qkv = w_qkv @ x; q,k,v heads of 64
    q = softmax_d(q) * 64**-0.5 ; k = softmax_n(k) ; v = v/4096
    ctx[h] = k[h] @ v[h].T ; out[h] = ctx[h].T @ q[h]
    y = w_out @ out + b_out ; LayerNorm_c(y) * g
Sharding: data-parallel over batch, 2 batch elements per core, no collectives.

Design (v5) — elementwise-bound rework of v4:
- kv computed TRANSPOSED per 128-token chunk in ONE PSUM bank (k||v, two
  N=512 matmuls); kexp via ACT Exp, vT via DVE copy. ksum rides as ones
  columns in the vT tiles.
- q natural; per-(head,n) colsums packed into ONE [128,512] PSUM bank via
  col-offset tile_position matmuls -> a single reciprocal_approx_fast +
  cast for 4 n-tiles (instead of 16 tiny [4,512] DVE ops).
- qbc broadcast via row-offset tile_position matmuls against the packed
  qsr bank; qn = qexp * qbc on DVE.
- stage C: y stays in PSUM until scaled. ACT Square drains y^2 to scratch
  (one [128,512] op per bank), DVE tensor_scalar+accum_out builds the
  variance columns, rstd = Exp(-0.5*Ln(var+eps)) on ACT (stays in the
  natural_log_exp table family -> NO ACT table reloads anywhere), final
  scale = DVE tensor_scalar reading PSUM directly into the DMA staging
  tile. Eliminates the separate y drain pass of v4.
- x loaded in 8 ordered 512-token pieces per batch on the (idle) GpSimd
  queue so the first kv matmul starts ~1.5us in, not 13us.
- PSUM budget (8 banks): pbig 3 {kv,q} + pc 2 {yT2,G} + pqs 2 {qs,qbc}
  + pctx 1 {ctx}.
- Host folds (exact): v/n into w_v; q-scale + LN mean-centering into w_out.
"""

import numpy as np

HEADS = 4
DIM_HEAD = 64
SCALE = DIM_HEAD ** -0.5
EPS = 1e-5
B, C, H, W = 16, 256, 64, 64
N = H * W  # 4096
HID = HEADS * DIM_HEAD  # 256
NCORES = 8
BPC = B // NCORES  # batches per core = 2

NCH = N // 128   # 32 token chunks of 128
NT = N // 512    # 8 n-tiles of 512 for the q stage
CT = C // 128    # 2 contraction tiles

_cache = {}


def _build_nc(G_IS_FULL=False):
    import concourse.bass as bass
    import concourse.tile as tile
    from concourse import bacc, masks, mybir

    f32 = mybir.dt.float32
    bf16 = mybir.dt.bfloat16
    AF = mybir.ActivationFunctionType
    OP = mybir.AluOpType

    # All ACT funcs used here (Exp, Ln, Square) live together in the
    # natural_log_exp_and_others table, but the table-load pass assigns each
    # func its first-containing table (exp -> exp_and_others, ln ->
    # natural_log), thrashing ACT_TABLE_LOADs on every rstd chain (~1.3us
    # each). Blank out every other table's func set (list order and hence
    # act_func_set ids stay intact) so one table serves the whole kernel.
    import concourse.hw_specs as hw_specs
    if not hasattr(bacc, "_orig_get_activation_tables"):
        bacc._orig_get_activation_tables = bacc.get_activation_tables

        def _tables_nle(arch):
            items = list(bacc._orig_get_activation_tables(arch).items())
            return {name: (funcs if name == "natural_log_exp_and_others" else set())
                    for name, funcs in items}

        bacc.get_activation_tables = _tables_nle

    nc = bacc.Bacc(None, target_bir_lowering=False, debug=False)
    x_ext = nc.declare_dram_parameter("x", [BPC, 128, CT, N], bf16, isOutput=False)
    wqkvT_ext = nc.declare_dram_parameter("wqkvT", [128, CT, 3 * HID], bf16, isOutput=False)
    woutcT_ext = nc.declare_dram_parameter("woutcT", [128, CT, C], bf16, isOutput=False)
    bc4_ext = nc.declare_dram_parameter("bc4", [1, C], bf16, isOutput=False)
    g_ext = nc.declare_dram_parameter("g", [1, C], bf16, isOutput=False)
    # row-selector patterns for the qbc broadcast matmuls (per qt), rows at
    # 32-offsets matching the packed qsr bank
    obc2_ext = nc.declare_dram_parameter("obc2", [128, 2, 128], bf16, isOutput=False)
    out_ext = nc.declare_dram_parameter("out", [BPC, N, C], bf16, isOutput=True)

    with tile.TileContext(nc) as tc:
        with (
            tc.tile_pool(name="wts", bufs=1) as wts,
            tc.tile_pool(name="xs", bufs=2) as xs_pool,
            tc.tile_pool(name="kex", bufs=6) as kex_pool,
            tc.tile_pool(name="qb", bufs=1) as qb_pool,
            tc.tile_pool(name="qnb", bufs=2) as qn_pool,
            tc.tile_pool(name="small", bufs=3) as small_pool,
            tc.tile_pool(name="scr", bufs=4) as scr_pool,
            tc.tile_pool(name="qsr", bufs=2) as qsr_pool,
            tc.tile_pool(name="stg", bufs=3) as stg_pool,
            tc.tile_pool(name="pbig", bufs=3, space="PSUM") as pbig,
            tc.tile_pool(name="pc", bufs=2, space="PSUM") as pc_pool,
            tc.tile_pool(name="pqs", bufs=2, space="PSUM") as pqs,
            tc.tile_pool(name="pctx", bufs=1, space="PSUM") as pctx,
        ):
            # ---- constants & weights (loaded once, sync queue) ----
            wqkvT3 = wts.tile([128, CT, 3 * HID], bf16, tag="wqkvT", name="wqkvT")
            nc.sync.dma_start(out=wqkvT3, in_=wqkvT_ext[:, :, :])
            woutcT3 = wts.tile([128, CT, C], bf16, tag="woutcT", name="woutcT")
            nc.sync.dma_start(out=woutcT3, in_=woutcT_ext[:, :, :])
            bc4_sb = wts.tile([1, C], bf16, tag="bc4", name="bc4")
            nc.sync.dma_start(out=bc4_sb, in_=bc4_ext[:, :])
            g_row = wts.tile([1, C], bf16, tag="grow", name="grow")
            nc.sync.dma_start(out=g_row, in_=g_ext[:, :])
            obc2sb = wts.tile([128, 2, 128], bf16, tag="obc2", name="obc2")
            nc.sync.dma_start(out=obc2sb, in_=obc2_ext[:, :, :])

            ones1x128 = wts.tile([1, 128], bf16, tag="ones1x128", name="ones1x128")
            nc.vector.memset(ones1x128, 1.0)
            # qs lhsT pair: col (2qt+hh) sums partitions hh*64..hh*64+63
            onesblk4 = []
            for which in range(2):
                ob = wts.tile([128, 4], bf16, tag=f"onesblk4{which}", name=f"onesblk4{which}")
                nc.vector.memset(ob, 0.0)
                nc.vector.memset(ob[0:64, 2 * which:2 * which + 1], 1.0)
                nc.vector.memset(ob[64:128, 2 * which + 1:2 * which + 2], 1.0)
                onesblk4.append(ob)
            eps_sb = wts.tile([128, 1], f32, tag="eps", name="eps")
            nc.vector.memset(eps_sb, EPS)

            ident = wts.tile([128, 128], bf16, tag="ident", name="ident")
            g_bc = wts.tile([128, C], bf16, tag="gbc", name="gbc")
            late = {"done": False}

            def build_late_consts():
                # deferred so the startup DMAs/PE aren't blocked by them
                if late["done"]:
                    return
                late["done"] = True
                masks.make_identity(nc, ident[:, :])
                if G_IS_FULL:
                    g_ps = pc_pool.tile([128, 512], f32, tag="c", name="gps")
                    nc.tensor.matmul(g_ps[:, 0:C], ones1x128, g_row,
                                     start=True, stop=True)
                    nc.vector.tensor_copy(out=g_bc, in_=g_ps[:, 0:C])

            # manual vT buffers holding 2 chunks each, with persistent ones
            # columns at 128/257/386/515 (the ksum column per head-pair)
            NVT = 4
            vts = []
            for i in range(NVT):
                vt = wts.tile([128, 516], bf16, tag=f"vt{i}", name=f"vt{i}")
                for j in range(4):
                    nc.vector.memset(vt[:, j * 129 + 128:j * 129 + 129], 1.0)
                vts.append(vt)

            # ---- load x: 8 ordered 512-token pieces per batch on the gpsimd
            # queue; piece 0 of batch 0 lands first so kv matmuls start early
            xs_all = []
            for b in range(BPC):
                xs3 = xs_pool.tile([128, CT, N], bf16, tag="x", name="x")
                xr = x_ext[b]
                for p8 in range(8):
                    psl = slice(p8 * 512, (p8 + 1) * 512)
                    nc.gpsimd.dma_start(out=xs3[:, :, psl], in_=xr[:, :, psl])
                xs_all.append([xs3[:, i] for i in range(CT)])

            # per-batch state carried between stages
            st = [dict() for _ in range(BPC)]

            def stage_AB(b, c_gen=None, at_cp1=None, at_end_ctx=None):
                """Interleaved kv+ctx (transposed) and q (natural) stages."""
                xs = xs_all[b]
                ctx_t = pctx.tile([128, 258], f32, tag="ctx", name="ctx")
                st[b]["ctx"] = ctx_t
                kexps = [None] * NCH
                qexp = [qb_pool.tile([128, N], bf16, tag=f"qexp{qt}", name=f"qexp{qt}")
                        for qt in range(2)]
                qn = [qn_pool.tile([128, N], bf16, tag=f"qn{qt}", name=f"qn{qt}")
                      for qt in range(2)]
                st[b]["qn"] = qn
                qsrs = [None, None]

                def kv_chunk(ch):
                    kv_ps = pbig.tile([128, 512], f32, tag="big", name="kv")
                    csl = slice(ch * 128, (ch + 1) * 128)
                    for ct in range(CT):
                        nc.tensor.matmul(
                            kv_ps, xs[ct][:, csl], wqkvT3[:, ct, HID:3 * HID],
                            start=(ct == 0), stop=(ct == CT - 1),
                            skip_group_check=True,
                        )
                    kexp = kex_pool.tile([128, 256], bf16, tag="kexp", name="kexp")
                    nc.scalar.activation(out=kexp, in_=kv_ps[:, 0:256], func=AF.Exp)
                    kexps[ch] = kexp
                    vt = vts[(ch // 2) % NVT]
                    vo = (ch % 2) * 258
                    vdst = vt[:, vo:vo + 258].rearrange("p (hp x) -> p hp x", hp=2)[:, :, 0:128]
                    vsrc = kv_ps[:, 256:512].rearrange("p (hp x) -> p hp x", hp=2)
                    nc.vector.tensor_copy(out=vdst, in_=vsrc)

                def ctx_mms(ch):
                    kex = kexps[ch]
                    vt = vts[(ch // 2) % NVT]
                    vo = (ch % 2) * 258
                    for hp in range(2):
                        nc.tensor.matmul(
                            ctx_t[:, hp * 129:(hp + 1) * 129],
                            kex[:, hp * 128:(hp + 1) * 128],
                            vt[:, vo + hp * 129:vo + (hp + 1) * 129],
                            start=(ch == 0 and hp == 0),
                            stop=(ch == NCH - 1 and hp == 1),
                            skip_group_check=True,
                        )

                def q_mms(nt):
                    nsl = slice(nt * 512, (nt + 1) * 512)
                    for qt in range(2):
                        q_ps = pbig.tile([128, 512], f32, tag="big", name="q")
                        for ct in range(CT):
                            nc.tensor.matmul(
                                q_ps,
                                wqkvT3[:, ct, qt * 128:(qt + 1) * 128],
                                xs[ct][:, nsl],
                                start=(ct == 0), stop=(ct == CT - 1),
                            )
                        nc.scalar.activation(out=qexp[qt][:, nsl], in_=q_ps,
                                             func=AF.Exp)

                def qs_burst(g):
                    """Pack colsums for n-tiles 4g..4g+3 into one PSUM bank at
                    32-partition offsets (col-offset tile_position), then one
                    reciprocal + one bf16 cast for the whole group."""
                    qs32 = pqs.tile([128, 512], f32, tag="qsb", name="qs32")
                    for i in range(4):
                        nt = 4 * g + i
                        nsl = slice(nt * 512, (nt + 1) * 512)
                        for qt in range(2):
                            nc.tensor.matmul(
                                qs32[32 * i:32 * i + 4, :],
                                onesblk4[qt], qexp[qt][:, nsl],
                                start=(qt == 0), stop=(qt == 1),
                                skip_group_check=True,
                                tile_position=(0, 32 * i),
                            )
                    qsrf = qsr_pool.tile([128, 512], f32, tag="qsrf", name="qsrf")
                    nc.vector.reciprocal_approx_fast(out=qsrf, in_=qs32)
                    qsr = qsr_pool.tile([128, 512], bf16, tag="qsr", name="qsr")
                    nc.vector.tensor_copy(out=qsr, in_=qsrf)
                    qsrs[g] = qsr

                def qbc_qn(nt):
                    g, i = nt // 4, nt % 4
                    qsr = qsrs[g]
                    nsl = slice(nt * 512, (nt + 1) * 512)
                    for qt in range(2):
                        qbc_ps = pqs.tile([128, 512], f32, tag="qsb", name="qbc")
                        nc.tensor.matmul(
                            qbc_ps,
                            obc2sb[32 * i:32 * i + 4, qt, :],
                            qsr[32 * i:32 * i + 4, :],
                            start=True, stop=True,
                            tile_position=(32 * i, 0),
                        )
                        nc.vector.tensor_mul(out=qn[qt][:, nsl],
                                             in0=qexp[qt][:, nsl], in1=qbc_ps)

                # software-pipelined emission
                for ch in range(NCH):
                    kv_chunk(ch)
                    if ch % 4 == 3:
                        q_mms(ch // 4)
                    if ch == 1 and at_cp1 is not None:
                        at_cp1()
                    if ch >= 2:
                        ctx_mms(ch - 2)
                    if ch == 17:
                        qs_burst(0)
                    if ch >= 19 and ch % 3 == 1 and (ch - 19) // 3 < 4:
                        qbc_qn((ch - 19) // 3)
                    if c_gen is not None and ch >= 4 and ch % 2 == 1:
                        next(c_gen, None)
                ctx_mms(NCH - 2)
                ctx_mms(NCH - 1)
                if at_end_ctx is not None:
                    at_end_ctx()
                qs_burst(1)
                for nt in range(4, 8):
                    qbc_qn(nt)

            def stage_G_pre(b):
                """ctx normalize on DVE (krecip + block-diag scale)."""
                ctx_t = st[b]["ctx"]
                krecip = small_pool.tile([128, 2], f32, tag="krecip", name="krecip")
                for hp in range(2):
                    nc.vector.reciprocal(out=krecip[:, hp:hp + 1],
                                         in_=ctx_t[:, hp * 129 + 128:hp * 129 + 129])
                ctx_sb = [small_pool.tile([128, 128], bf16, tag=f"ctxsb{i}", name=f"ctxsb{i}")
                          for i in range(2)]
                for hp in range(2):
                    nc.vector.memset(ctx_sb[hp], 0.0)
                    for hh in range(2):
                        s = slice(hh * 64, hh * 64 + 64)
                        nc.vector.tensor_scalar(
                            out=ctx_sb[hp][s, s],
                            in0=ctx_t[s, hp * 129 + hh * 64:hp * 129 + hh * 64 + 64],
                            scalar1=krecip[s, hp:hp + 1],
                            scalar2=None,
                            op0=OP.mult,
                        )
                st[b]["ctx_sb"] = ctx_sb

            def stage_G_fin(b):
                """transpose + G = ctxT @ woutT (+ bias fold) on the PE."""
                ctx_sb = st[b]["ctx_sb"]
                G_sb = small_pool.tile([128, 2, C], bf16, tag="G", name="G")
                st[b]["G"] = G_sb
                for hp in range(2):
                    ctxT_ps = pc_pool.tile([128, 512], f32, tag="c", name="ctxT")
                    ctxT_ps_bf = ctxT_ps.bitcast(bf16)[:, 0:128]
                    nc.tensor.transpose(ctxT_ps_bf, ctx_sb[hp], ident)
                    ctxT_sb = small_pool.tile([128, 128], bf16, tag=f"ctxT{hp}", name=f"ctxT{hp}")
                    nc.vector.tensor_copy(out=ctxT_sb, in_=ctxT_ps_bf)
                    G_ps = pc_pool.tile([128, 512], f32, tag="c", name="Gps")
                    nc.tensor.matmul(G_ps[:, 0:C], ctxT_sb, woutcT3[:, hp],
                                     start=True, stop=False)
                    nc.tensor.matmul(G_ps[:, 0:C], ones1x128, bc4_sb,
                                     start=False, stop=True)
                    nc.vector.tensor_copy(out=G_sb[:, hp], in_=G_ps[:, 0:C])

            def stage_C(b):
                """yT = qn^T-chunks @ G (2 chunks per bank); y held in PSUM
                until the LN scale: ACT Square drains y^2, DVE accumulates
                variance columns, rstd = Exp(-0.5 Ln(var+eps)) on ACT, final
                scale reads PSUM straight into the bf16 staging tile."""
                qn = st[b]["qn"]
                G_sb = st[b]["G"]
                s2_all = small_pool.tile([128, NCH], f32, tag="s2", name="s2")
                rstd_all = small_pool.tile([128, NCH], f32, tag="rstd", name="rstd")
                outr = out_ext[b].rearrange("(c p) f -> p c f", p=128)
                stg = None
                for g2 in range(NCH // 2):
                    ch0 = g2 * 2
                    if ch0 % 8 == 0:
                        stg = stg_pool.tile([128, 8, C], bf16, tag="stg", name="stg")
                    yT2_ps = pc_pool.tile([128, 512], f32, tag="c", name="yT2")
                    for half in range(2):
                        ch = ch0 + half
                        csl = slice(ch * 128, (ch + 1) * 128)
                        for qt in range(2):
                            nc.tensor.matmul(
                                yT2_ps[:, half * C:(half + 1) * C],
                                qn[qt][:, csl], G_sb[:, qt],
                                start=(qt == 0), stop=(qt == 1),
                                skip_group_check=True,
                            )
                    # y^2 drain (ACT, one [128,512] op, exp-family table)
                    y2_sb = scr_pool.tile([128, 512], bf16, tag="y2", name="y2")
                    nc.scalar.activation(out=y2_sb, in_=yT2_ps, func=AF.Square)
                    # variance columns (DVE 4x-mode tensor_scalar + accum)
                    for half in range(2):
                        ch = ch0 + half
                        garb = scr_pool.tile([128, C], bf16, tag="garb", name="garb")
                        nc.vector.tensor_scalar(
                            out=garb, in0=y2_sb[:, half * C:(half + 1) * C],
                            scalar1=1.0, scalar2=None, op0=OP.mult,
                            op1=OP.add,
                            accum_out=s2_all[:, ch:ch + 1],
                        )
                    # rstd = exp(-0.5 * ln(s2/C + eps))  (no table switch)
                    lnv = scr_pool.tile([128, 2], f32, tag="lnv", name="lnv")
                    nc.scalar.activation(out=lnv, in_=s2_all[:, ch0:ch0 + 2],
                                         func=AF.Ln, bias=eps_sb, scale=1.0 / C)
                    nc.scalar.activation(out=rstd_all[:, ch0:ch0 + 2], in_=lnv,
                                         func=AF.Exp, scale=-0.5)
                    # final scale straight from PSUM -> staging (frees bank)
                    for half in range(2):
                        ch = ch0 + half
                        nc.vector.tensor_scalar(
                            out=stg[:, ch % 8],
                            in0=yT2_ps[:, half * C:(half + 1) * C],
                            scalar1=rstd_all[:, ch:ch + 1], scalar2=None,
                            op0=OP.mult)
                        if G_IS_FULL:
                            nc.gpsimd.tensor_mul(out=stg[:, ch % 8],
                                                 in0=stg[:, ch % 8], in1=g_bc)
                    if ch0 % 4 == 2:
                        g4 = ch0 // 4
                        nc.sync.dma_start(
                            out=outr[:, g4 * 4:(g4 + 1) * 4],
                            in_=stg[:, (g4 % 2) * 4:(g4 % 2) * 4 + 4])
                    yield

            # emission order: both batches' matmul-dense stages back-to-back
            stage_AB(0)
            build_late_consts()
            stage_G_pre(0)
            c0 = stage_C(0)
            stage_AB(1, c_gen=c0, at_cp1=lambda: stage_G_fin(0))
            stage_G_pre(1)
            for _ in c0:
                pass
            stage_G_fin(1)
            for _ in stage_C(1):
                pass

    nc.compile()
    return nc


def _prep_weights(w_qkv, w_out, b_out, g):
    import ml_dtypes
    w_qkv = np.asarray(w_qkv, dtype=np.float64)
    w_out = np.asarray(w_out, dtype=np.float64)
    b_out = np.asarray(b_out, dtype=np.float64)
    g64 = np.asarray(g, dtype=np.float64)
    wq = w_qkv.copy()
    wq[2 * HID:3 * HID, :] /= N          # fold v/n
    wqkvT = wq.T.reshape(CT, 128, 3 * HID).transpose(1, 0, 2)
    wqkvT = np.ascontiguousarray(wqkvT).astype(ml_dtypes.bfloat16)
    wo = w_out * SCALE                    # fold q scale
    wo = wo - wo.mean(axis=0, keepdims=True)  # fold LN mean-centering
    woutcT = wo.T.reshape(CT, 128, C).transpose(1, 0, 2)
    woutcT = np.ascontiguousarray(woutcT).astype(ml_dtypes.bfloat16)
    bc4 = ((b_out - b_out.mean()) / 4.0).astype(ml_dtypes.bfloat16).reshape(1, C)
    g_row = g64.astype(ml_dtypes.bfloat16).reshape(1, C)
    return wqkvT, woutcT, bc4, g_row


def _make_in_maps(x, w_qkv, w_out, b_out, g):
    import ml_dtypes
    xf = np.asarray(x, dtype=np.float32).reshape(B, CT, 128, N).transpose(0, 2, 1, 3)
    xf = np.ascontiguousarray(xf).astype(ml_dtypes.bfloat16)
    wqkvT, woutcT, bc4, g_row = _prep_weights(w_qkv, w_out, b_out, g)
    # qbc row-selector: obc2[32*i + 2*qt + hh, qt, 64*hh : 64*hh+64] = 1
    obc2 = np.zeros((128, 2, 128), dtype=ml_dtypes.bfloat16)
    for qt in range(2):
        for i in range(4):
            for hh in range(2):
                obc2[32 * i + 2 * qt + hh, qt, 64 * hh:64 * hh + 64] = 1.0
    in_maps = []
    for i in range(NCORES):
        in_maps.append({
            "x": np.ascontiguousarray(xf[i * BPC:(i + 1) * BPC]),
            "wqkvT": wqkvT,
            "woutcT": woutcT,
            "bc4": bc4,
            "g": g_row,
            "obc2": obc2,
        })
    return in_maps


def kernel(x, w_qkv, w_out, b_out, g):
    from concourse.bass_utils import run_bass_kernel_spmd

    g_full = not np.allclose(np.asarray(g, dtype=np.float64), 1.0)
    key = f"nc{int(g_full)}"
    if key not in _cache:
        _cache[key] = _build_nc(G_IS_FULL=g_full)
    nc = _cache[key]

    in_maps = _make_in_maps(x, w_qkv, w_out, b_out, g)
    res = run_bass_kernel_spmd(nc, in_maps, core_ids=list(range(NCORES)))
    outs = [res.results[i]["out"] for i in range(NCORES)]
    yT = np.concatenate(outs, axis=0).astype(np.float32)  # [B, N, C]
    y = np.ascontiguousarray(yT.transpose(0, 2, 1)).reshape(B, C, H, W)
    return y


# revision 14
# speedup vs baseline: 1.6965x; 1.1533x over previous
"""Trainium2 Bass kernel for the linear-attention block (nn_Attention).

Per batch element (x: [256, 4096] after flattening h*w):
    qkv = w_qkv @ x; q,k,v heads of 64
    q = softmax_d(q) * 64**-0.5 ; k = softmax_n(k) ; v = v/4096
    ctx[h] = k[h] @ v[h].T ; out[h] = ctx[h].T @ q[h]
    y = w_out @ out + b_out ; LayerNorm_c(y) * g
Sharding: data-parallel over batch, 2 batch elements per core, no collectives.

Design (v5) — elementwise-bound rework of v4:
- kv computed TRANSPOSED per 128-token chunk in ONE PSUM bank (k||v, two
  N=512 matmuls); kexp via ACT Exp, vT via DVE copy. ksum rides as ones
  columns in the vT tiles.
- q natural; per-(head,n) colsums packed into ONE [128,512] PSUM bank via
  col-offset tile_position matmuls -> a single reciprocal_approx_fast +
  cast for 4 n-tiles (instead of 16 tiny [4,512] DVE ops).
- qbc broadcast via row-offset tile_position matmuls against the packed
  qsr bank; qn = qexp * qbc on DVE.
- stage C: y stays in PSUM until scaled. ACT Square drains y^2 to scratch
  (one [128,512] op per bank), DVE tensor_scalar+accum_out builds the
  variance columns, rstd = Exp(-0.5*Ln(var+eps)) on ACT (stays in the
  natural_log_exp table family -> NO ACT table reloads anywhere), final
  scale = DVE tensor_scalar reading PSUM directly into the DMA staging
  tile. Eliminates the separate y drain pass of v4.
- x loaded in 8 ordered 512-token pieces per batch on the (idle) GpSimd
  queue so the first kv matmul starts ~1.5us in, not 13us.
- PSUM budget (8 banks): pbig 3 {kv,q} + pc 2 {yT2,G} + pqs 2 {qs,qbc}
  + pctx 1 {ctx}.
- Host folds (exact): v/n into w_v; q-scale + LN mean-centering into w_out.
"""

import numpy as np

HEADS = 4
DIM_HEAD = 64
SCALE = DIM_HEAD ** -0.5
EPS = 1e-5
B, C, H, W = 16, 256, 64, 64
N = H * W  # 4096
HID = HEADS * DIM_HEAD  # 256
NCORES = 8
BPC = B // NCORES  # batches per core = 2

NCH = N // 128   # 32 token chunks of 128
NT = N // 512    # 8 n-tiles of 512 for the q stage
CT = C // 128    # 2 contraction tiles

_cache = {}


def _build_nc(G_IS_FULL=False):
    import concourse.bass as bass
    import concourse.tile as tile
    from concourse import bacc, masks, mybir

    f32 = mybir.dt.float32
    bf16 = mybir.dt.bfloat16
    AF = mybir.ActivationFunctionType
    OP = mybir.AluOpType

    # All ACT funcs used here (Exp, Ln, Square) live together in the
    # natural_log_exp_and_others table, but the table-load pass assigns each
    # func its first-containing table (exp -> exp_and_others, ln ->
    # natural_log), thrashing ACT_TABLE_LOADs on every rstd chain (~1.3us
    # each). Blank out every other table's func set (list order and hence
    # act_func_set ids stay intact) so one table serves the whole kernel.
    import concourse.hw_specs as hw_specs
    if not hasattr(bacc, "_orig_get_activation_tables"):
        bacc._orig_get_activation_tables = bacc.get_activation_tables

        def _tables_nle(arch):
            items = list(bacc._orig_get_activation_tables(arch).items())
            return {name: (funcs if name == "natural_log_exp_and_others" else set())
                    for name, funcs in items}

        bacc.get_activation_tables = _tables_nle

    nc = bacc.Bacc(None, target_bir_lowering=False, debug=False)
    x_ext = nc.declare_dram_parameter("x", [BPC, 128, CT, N], bf16, isOutput=False)
    wqkvT_ext = nc.declare_dram_parameter("wqkvT", [128, CT, 3 * HID], bf16, isOutput=False)
    woutcT_ext = nc.declare_dram_parameter("woutcT", [128, CT, C], bf16, isOutput=False)
    bc4_ext = nc.declare_dram_parameter("bc4", [1, C], bf16, isOutput=False)
    g_ext = nc.declare_dram_parameter("g", [1, C], bf16, isOutput=False)
    # row-selector patterns for the qbc broadcast matmuls (per qt), rows at
    # 32-offsets matching the packed qsr bank
    obc2_ext = nc.declare_dram_parameter("obc2", [128, 2, 128], bf16, isOutput=False)
    out_ext = nc.declare_dram_parameter("out", [BPC, N, C], bf16, isOutput=True)

    with tile.TileContext(nc) as tc:
        with (
            tc.tile_pool(name="wts", bufs=1) as wts,
            tc.tile_pool(name="xs", bufs=2) as xs_pool,
            tc.tile_pool(name="kex", bufs=6) as kex_pool,
            tc.tile_pool(name="qb", bufs=1) as qb_pool,
            tc.tile_pool(name="qnb", bufs=2) as qn_pool,
            tc.tile_pool(name="small", bufs=3) as small_pool,
            tc.tile_pool(name="scr", bufs=4) as scr_pool,
            tc.tile_pool(name="qsr", bufs=2) as qsr_pool,
            tc.tile_pool(name="stg", bufs=3) as stg_pool,
            tc.tile_pool(name="pbig", bufs=3, space="PSUM") as pbig,
            tc.tile_pool(name="pc", bufs=2, space="PSUM") as pc_pool,
            tc.tile_pool(name="pqs", bufs=2, space="PSUM") as pqs,
            tc.tile_pool(name="pctx", bufs=1, space="PSUM") as pctx,
        ):
            # ---- constants & weights (loaded once, sync queue) ----
            wqkvT3 = wts.tile([128, CT, 3 * HID], bf16, tag="wqkvT", name="wqkvT")
            nc.sync.dma_start(out=wqkvT3, in_=wqkvT_ext[:, :, :])
            woutcT3 = wts.tile([128, CT, C], bf16, tag="woutcT", name="woutcT")
            nc.sync.dma_start(out=woutcT3, in_=woutcT_ext[:, :, :])
            bc4_sb = wts.tile([1, C], bf16, tag="bc4", name="bc4")
            nc.sync.dma_start(out=bc4_sb, in_=bc4_ext[:, :])
            g_row = wts.tile([1, C], bf16, tag="grow", name="grow")
            nc.sync.dma_start(out=g_row, in_=g_ext[:, :])
            obc2sb = wts.tile([128, 2, 128], bf16, tag="obc2", name="obc2")
            nc.sync.dma_start(out=obc2sb, in_=obc2_ext[:, :, :])

            ones1x128 = wts.tile([1, 128], bf16, tag="ones1x128", name="ones1x128")
            nc.vector.memset(ones1x128, 1.0)
            # qs lhsT pair: col (2qt+hh) sums partitions hh*64..hh*64+63
            onesblk4 = []
            for which in range(2):
                ob = wts.tile([128, 4], bf16, tag=f"onesblk4{which}", name=f"onesblk4{which}")
                nc.vector.memset(ob, 0.0)
                nc.vector.memset(ob[0:64, 2 * which:2 * which + 1], 1.0)
                nc.vector.memset(ob[64:128, 2 * which + 1:2 * which + 2], 1.0)
                onesblk4.append(ob)
            eps_sb = wts.tile([128, 1], f32, tag="eps", name="eps")
            nc.vector.memset(eps_sb, EPS)

            ident = wts.tile([128, 128], bf16, tag="ident", name="ident")
            g_bc = wts.tile([128, C], bf16, tag="gbc", name="gbc")
            late = {"done": False}

            def build_late_consts():
                # deferred so the startup DMAs/PE aren't blocked by them
                if late["done"]:
                    return
                late["done"] = True
                masks.make_identity(nc, ident[:, :])
                if G_IS_FULL:
                    g_ps = pc_pool.tile([128, 512], f32, tag="c", name="gps")
                    nc.tensor.matmul(g_ps[:, 0:C], ones1x128, g_row,
                                     start=True, stop=True)
                    nc.vector.tensor_copy(out=g_bc, in_=g_ps[:, 0:C])

            # manual vT buffers holding 2 chunks each, with persistent ones
            # columns at 128/257/386/515 (the ksum column per head-pair)
            NVT = 4
            vts = []
            for i in range(NVT):
                vt = wts.tile([128, 516], bf16, tag=f"vt{i}", name=f"vt{i}")
                for j in range(4):
                    nc.vector.memset(vt[:, j * 129 + 128:j * 129 + 129], 1.0)
                vts.append(vt)

            # ---- load x: 8 ordered 512-token pieces per batch on the gpsimd
            # queue; piece 0 of batch 0 lands first so kv matmuls start early
            xs_all = []
            for b in range(BPC):
                xs3 = xs_pool.tile([128, CT, N], bf16, tag="x", name="x")
                xr = x_ext[b]
                for p8 in range(8):
                    psl = slice(p8 * 512, (p8 + 1) * 512)
                    nc.gpsimd.dma_start(out=xs3[:, :, psl], in_=xr[:, :, psl])
                xs_all.append([xs3[:, i] for i in range(CT)])

            # per-batch state carried between stages
            st = [dict() for _ in range(BPC)]

            def stage_AB(b, c_gen=None, at_cp1=None, at_end_ctx=None):
                """Interleaved kv+ctx (transposed) and q (natural) stages."""
                xs = xs_all[b]
                ctx_t = pctx.tile([128, 258], f32, tag="ctx", name="ctx")
                st[b]["ctx"] = ctx_t
                kexps = [None] * NCH
                qexp = [qb_pool.tile([128, N], bf16, tag=f"qexp{qt}", name=f"qexp{qt}")
                        for qt in range(2)]
                qn = [qn_pool.tile([128, N], bf16, tag=f"qn{qt}", name=f"qn{qt}")
                      for qt in range(2)]
                st[b]["qn"] = qn
                qsrs = [None, None]

                def kv_chunk(ch):
                    kv_ps = pbig.tile([128, 512], f32, tag="big", name="kv")
                    csl = slice(ch * 128, (ch + 1) * 128)
                    for ct in range(CT):
                        nc.tensor.matmul(
                            kv_ps, xs[ct][:, csl], wqkvT3[:, ct, HID:3 * HID],
                            start=(ct == 0), stop=(ct == CT - 1),
                            skip_group_check=True,
                        )
                    kexp = kex_pool.tile([128, 256], bf16, tag="kexp", name="kexp")
                    nc.scalar.activation(out=kexp, in_=kv_ps[:, 0:256], func=AF.Exp)
                    kexps[ch] = kexp
                    vt = vts[(ch // 2) % NVT]
                    vo = (ch % 2) * 258
                    vdst = vt[:, vo:vo + 258].rearrange("p (hp x) -> p hp x", hp=2)[:, :, 0:128]
                    vsrc = kv_ps[:, 256:512].rearrange("p (hp x) -> p hp x", hp=2)
                    nc.vector.tensor_copy(out=vdst, in_=vsrc)

                def ctx_mms(ch):
                    kex = kexps[ch]
                    vt = vts[(ch // 2) % NVT]
                    vo = (ch % 2) * 258
                    for hp in range(2):
                        nc.tensor.matmul(
                            ctx_t[:, hp * 129:(hp + 1) * 129],
                            kex[:, hp * 128:(hp + 1) * 128],
                            vt[:, vo + hp * 129:vo + (hp + 1) * 129],
                            start=(ch == 0 and hp == 0),
                            stop=(ch == NCH - 1 and hp == 1),
                            skip_group_check=True,
                        )

                def q_mms(nt):
                    nsl = slice(nt * 512, (nt + 1) * 512)
                    for qt in range(2):
                        q_ps = pbig.tile([128, 512], f32, tag="big", name="q")
                        for ct in range(CT):
                            nc.tensor.matmul(
                                q_ps,
                                wqkvT3[:, ct, qt * 128:(qt + 1) * 128],
                                xs[ct][:, nsl],
                                start=(ct == 0), stop=(ct == CT - 1),
                            )
                        nc.scalar.activation(out=qexp[qt][:, nsl], in_=q_ps,
                                             func=AF.Exp)

                def qs_burst(g):
                    """Pack colsums for n-tiles 4g..4g+3 into one PSUM bank at
                    32-partition offsets (col-offset tile_position), then one
                    reciprocal + one bf16 cast for the whole group."""
                    qs32 = pqs.tile([128, 512], f32, tag="qsb", name="qs32")
                    for i in range(4):
                        nt = 4 * g + i
                        nsl = slice(nt * 512, (nt + 1) * 512)
                        for qt in range(2):
                            nc.tensor.matmul(
                                qs32[32 * i:32 * i + 4, :],
                                onesblk4[qt], qexp[qt][:, nsl],
                                start=(qt == 0), stop=(qt == 1),
                                skip_group_check=True,
                                tile_position=(0, 32 * i),
                            )
                    qsrf = qsr_pool.tile([128, 512], f32, tag="qsrf", name="qsrf")
                    nc.vector.reciprocal_approx_fast(out=qsrf, in_=qs32)
                    qsr = qsr_pool.tile([128, 512], bf16, tag="qsr", name="qsr")
                    nc.vector.tensor_copy(out=qsr, in_=qsrf)
                    qsrs[g] = qsr

                def qbc_qn(nt):
                    g, i = nt // 4, nt % 4
                    qsr = qsrs[g]
                    nsl = slice(nt * 512, (nt + 1) * 512)
                    for qt in range(2):
                        qbc_ps = pqs.tile([128, 512], f32, tag="qsb", name="qbc")
                        nc.tensor.matmul(
                            qbc_ps,
                            obc2sb[32 * i:32 * i + 4, qt, :],
                            qsr[32 * i:32 * i + 4, :],
                            start=True, stop=True,
                            tile_position=(32 * i, 0),
                        )
                        nc.vector.tensor_mul(out=qn[qt][:, nsl],
                                             in0=qexp[qt][:, nsl], in1=qbc_ps)

                # software-pipelined emission
                for ch in range(NCH):
                    kv_chunk(ch)
                    if ch % 4 == 3:
                        q_mms(ch // 4)
                    if ch == 1 and at_cp1 is not None:
                        at_cp1()
                    if ch >= 2:
                        ctx_mms(ch - 2)
                    if ch == 17:
                        qs_burst(0)
                    if ch >= 19 and ch % 3 == 1 and (ch - 19) // 3 < 4:
                        qbc_qn((ch - 19) // 3)
                    if c_gen is not None and ch >= 4 and ch % 2 == 1:
                        next(c_gen, None)
                ctx_mms(NCH - 2)
                ctx_mms(NCH - 1)
                if at_end_ctx is not None:
                    at_end_ctx()
                qs_burst(1)
                for nt in range(4, 8):
                    qbc_qn(nt)

            def stage_G_pre(b):
                """ctx normalize on DVE (krecip + block-diag scale)."""
                ctx_t = st[b]["ctx"]
                krecip = small_pool.tile([128, 2], f32, tag="krecip", name="krecip")
                for hp in range(2):
                    nc.vector.reciprocal(out=krecip[:, hp:hp + 1],
                                         in_=ctx_t[:, hp * 129 + 128:hp * 129 + 129])
                ctx_sb = [small_pool.tile([128, 128], bf16, tag=f"ctxsb{i}", name=f"ctxsb{i}")
                          for i in range(2)]
                for hp in range(2):
                    nc.vector.memset(ctx_sb[hp], 0.0)
                    for hh in range(2):
                        s = slice(hh * 64, hh * 64 + 64)
                        nc.vector.tensor_scalar(
                            out=ctx_sb[hp][s, s],
                            in0=ctx_t[s, hp * 129 + hh * 64:hp * 129 + hh * 64 + 64],
                            scalar1=krecip[s, hp:hp + 1],
                            scalar2=None,
                            op0=OP.mult,
                        )
                st[b]["ctx_sb"] = ctx_sb

            def stage_G_fin(b):
                """transpose + G = ctxT @ woutT (+ bias fold) on the PE."""
                ctx_sb = st[b]["ctx_sb"]
                G_sb = small_pool.tile([128, 2, C], bf16, tag="G", name="G")
                st[b]["G"] = G_sb
                for hp in range(2):
                    ctxT_ps = pc_pool.tile([128, 512], f32, tag="c", name="ctxT")
                    ctxT_ps_bf = ctxT_ps.bitcast(bf16)[:, 0:128]
                    nc.tensor.transpose(ctxT_ps_bf, ctx_sb[hp], ident)
                    ctxT_sb = small_pool.tile([128, 128], bf16, tag=f"ctxT{hp}", name=f"ctxT{hp}")
                    nc.vector.tensor_copy(out=ctxT_sb, in_=ctxT_ps_bf)
                    G_ps = pc_pool.tile([128, 512], f32, tag="c", name="Gps")
                    nc.tensor.matmul(G_ps[:, 0:C], ctxT_sb, woutcT3[:, hp],
                                     start=True, stop=False)
                    nc.tensor.matmul(G_ps[:, 0:C], ones1x128, bc4_sb,
                                     start=False, stop=True)
                    nc.vector.tensor_copy(out=G_sb[:, hp], in_=G_ps[:, 0:C])

            def stage_C(b, use_big=False):
                """yT = qn^T-chunks @ G (2 chunks per bank); y held in PSUM
                until the LN scale: DVE STT squares+accumulates straight from
                PSUM, rstd = Exp(-0.5 Ln(var+eps)) on ACT, final scale = ACT
                Copy-with-scale from PSUM into the bf16 staging tile.
                use_big: alternate banks between both PSUM pools (for the
                batch-1 tail, when the AB pools are idle) to deepen the
                pipeline."""
                qn = st[b]["qn"]
                G_sb = st[b]["G"]
                rstd_all = small_pool.tile([128, NCH], f32, tag="rstd", name="rstd")
                outr = out_ext[b].rearrange("(c p) f -> p c f", p=128)
                stg = None
                for g2 in range(NCH // 2):
                    ch0 = g2 * 2
                    if ch0 % 8 == 0:
                        stg = stg_pool.tile([128, 8, C], bf16, tag="stg", name="stg")
                    if use_big and g2 % 2 == 1:
                        yT2_ps = pbig.tile([128, 512], f32, tag="big", name="yT2b")
                    else:
                        yT2_ps = pc_pool.tile([128, 512], f32, tag="c", name="yT2")
                    for half in range(2):
                        ch = ch0 + half
                        csl = slice(ch * 128, (ch + 1) * 128)
                        for qt in range(2):
                            nc.tensor.matmul(
                                yT2_ps[:, half * C:(half + 1) * C],
                                qn[qt][:, csl], G_sb[:, qt],
                                start=(qt == 0), stop=(qt == 1),
                                skip_group_check=True,
                            )
                    # variance via bn_stats/bn_aggr (one PSUM read per chunk,
                    # exact mean+var; mean is structurally 0 from the fold)
                    bnout = scr_pool.tile([128, 2, 6], f32, tag="bn", name="bn")
                    vagg = scr_pool.tile([128, 2, 2], f32, tag="vagg", name="vagg")
                    for half in range(2):
                        nc.vector.bn_stats(out=bnout[:, half],
                                           in_=yT2_ps[:, half * C:(half + 1) * C])
                        nc.vector.bn_aggr(out=vagg[:, half], in_=bnout[:, half])
                    # rstd = exp(-0.5 * ln(var + eps))  (no table switch)
                    lnv = scr_pool.tile([128, 2], f32, tag="lnv", name="lnv")
                    nc.scalar.activation(out=lnv, in_=vagg[:, :, 1],
                                         func=AF.Ln, bias=eps_sb, scale=1.0)
                    nc.scalar.activation(out=rstd_all[:, ch0:ch0 + 2], in_=lnv,
                                         func=AF.Exp, scale=-0.5)
                    # final scale straight from PSUM -> staging (frees bank)
                    for half in range(2):
                        ch = ch0 + half
                        nc.scalar.activation(
                            out=stg[:, ch % 8],
                            in_=yT2_ps[:, half * C:(half + 1) * C],
                            func=AF.Copy,
                            scale=rstd_all[:, ch:ch + 1])
                        if G_IS_FULL:
                            nc.gpsimd.tensor_mul(out=stg[:, ch % 8],
                                                 in0=stg[:, ch % 8], in1=g_bc)
                    if ch0 % 4 == 2:
                        g4 = ch0 // 4
                        nc.sync.dma_start(
                            out=outr[:, g4 * 4:(g4 + 1) * 4],
                            in_=stg[:, (g4 % 2) * 4:(g4 % 2) * 4 + 4])
                    yield

            # emission order: both batches' matmul-dense stages back-to-back
            stage_AB(0)
            build_late_consts()
            stage_G_pre(0)
            c0 = stage_C(0)
            stage_AB(1, c_gen=c0, at_cp1=lambda: stage_G_fin(0))
            stage_G_pre(1)
            for _ in c0:
                pass
            stage_G_fin(1)
            for _ in stage_C(1, use_big=True):
                pass

    nc.compile()
    return nc


def _prep_weights(w_qkv, w_out, b_out, g):
    import ml_dtypes
    w_qkv = np.asarray(w_qkv, dtype=np.float64)
    w_out = np.asarray(w_out, dtype=np.float64)
    b_out = np.asarray(b_out, dtype=np.float64)
    g64 = np.asarray(g, dtype=np.float64)
    wq = w_qkv.copy()
    wq[2 * HID:3 * HID, :] /= N          # fold v/n
    wqkvT = wq.T.reshape(CT, 128, 3 * HID).transpose(1, 0, 2)
    wqkvT = np.ascontiguousarray(wqkvT).astype(ml_dtypes.bfloat16)
    wo = w_out * SCALE                    # fold q scale
    wo = wo - wo.mean(axis=0, keepdims=True)  # fold LN mean-centering
    woutcT = wo.T.reshape(CT, 128, C).transpose(1, 0, 2)
    woutcT = np.ascontiguousarray(woutcT).astype(ml_dtypes.bfloat16)
    bc4 = ((b_out - b_out.mean()) / 4.0).astype(ml_dtypes.bfloat16).reshape(1, C)
    g_row = g64.astype(ml_dtypes.bfloat16).reshape(1, C)
    return wqkvT, woutcT, bc4, g_row


def _make_in_maps(x, w_qkv, w_out, b_out, g):
    import ml_dtypes
    xf = np.asarray(x, dtype=np.float32).reshape(B, CT, 128, N).transpose(0, 2, 1, 3)
    xf = np.ascontiguousarray(xf).astype(ml_dtypes.bfloat16)
    wqkvT, woutcT, bc4, g_row = _prep_weights(w_qkv, w_out, b_out, g)
    # qbc row-selector: obc2[32*i + 2*qt + hh, qt, 64*hh : 64*hh+64] = 1
    obc2 = np.zeros((128, 2, 128), dtype=ml_dtypes.bfloat16)
    for qt in range(2):
        for i in range(4):
            for hh in range(2):
                obc2[32 * i + 2 * qt + hh, qt, 64 * hh:64 * hh + 64] = 1.0
    in_maps = []
    for i in range(NCORES):
        in_maps.append({
            "x": np.ascontiguousarray(xf[i * BPC:(i + 1) * BPC]),
            "wqkvT": wqkvT,
            "woutcT": woutcT,
            "bc4": bc4,
            "g": g_row,
            "obc2": obc2,
        })
    return in_maps


def kernel(x, w_qkv, w_out, b_out, g):
    from concourse.bass_utils import run_bass_kernel_spmd

    g_full = not np.allclose(np.asarray(g, dtype=np.float64), 1.0)
    key = f"nc{int(g_full)}"
    if key not in _cache:
        _cache[key] = _build_nc(G_IS_FULL=g_full)
    nc = _cache[key]

    in_maps = _make_in_maps(x, w_qkv, w_out, b_out, g)
    res = run_bass_kernel_spmd(nc, in_maps, core_ids=list(range(NCORES)))
    outs = [res.results[i]["out"] for i in range(NCORES)]
    yT = np.concatenate(outs, axis=0).astype(np.float32)  # [B, N, C]
    y = np.ascontiguousarray(yT.transpose(0, 2, 1)).reshape(B, C, H, W)
    return y
